# revision 4
# baseline (speedup 1.0000x reference)
"""GNN message-passing (gather + segment-sum) Trainium2 kernel.

Strategy (dst-owner sharding, no collectives):
  - Core c owns output nodes [c*NPC, (c+1)*NPC).
  - Host counting-sorts edges into (core, bucket-group, src-block, bucket)
    sub-lists, pads each (bucket, src-block) sub-list to a multiple of 128
    edges (identical padded layout on every core -> one SPMD program).
  - Device per core:
      dma_gather x[src] rows (int16 block-local indices, 4 blocks of 25000
      rows) -> SBUF staging [128 edges, chunk, 64];
      one-hot of bucket-local dst via DVE is_equal against an iota row;
      PE matmul  psum[64 feats, 128 nodes] += msgs^T @ onehot  accumulated
      over a bucket's chunks; ACT copies psum -> SBUF out staging;
      one DMA of [64, NB*128] partial to HBM.
  - Host concatenates the 8 [64, 12500] shards -> [100000, 64].
"""

import sys

for _p in ("/opt/trn_rl_repo", "/root/.axon_site/_ro/trn_rl_repo"):
    if _p not in sys.path:
        sys.path.append(_p)

import numpy as np

from concourse import bass, mybir, tile, bacc
from concourse.bass_utils import run_bass_kernel_spmd

P = 128


def full_cfg():
    return dict(N=100000, D=64, E=1200000, CORES=8, BLOCK_ROWS=25000, GROUP=4)


def make_layout(edge_index, cfg):
    """Counting-sort edges into the padded SPMD layout.

    Returns (Cmat, meta, per-core arrays).
    """
    N, CORES, BLOCK_ROWS, GROUP = cfg["N"], cfg["CORES"], cfg["BLOCK_ROWS"], cfg["GROUP"]
    NPC = N // CORES
    NB = -(-NPC // P)                       # buckets per core
    NBLK = -(-N // BLOCK_ROWS)              # src blocks
    NG = -(-NB // GROUP)                    # bucket groups

    src = np.asarray(edge_index[0], dtype=np.int64)
    dst = np.asarray(edge_index[1], dtype=np.int64)
    E = src.shape[0]

    core = dst // NPC
    dstl = dst - core * NPC
    bucket = dstl >> 7
    din = (dstl & 127).astype(np.float32)
    blk = src // BLOCK_ROWS
    srcl = (src - blk * BLOCK_ROWS).astype(np.int16)
    g = bucket // GROUP
    bing = bucket - g * GROUP

    # per-(core,bucket,blk) counts -> shared padded chunk counts
    cid = (core * NB + bucket) * NBLK + blk
    n = np.bincount(cid, minlength=CORES * NB * NBLK).reshape(CORES, NB, NBLK)
    Cmat = -(-n.max(axis=0) // P)           # [NB, NBLK] chunks
    Cmat[:, 0] = np.maximum(Cmat[:, 0], 1)  # every bucket gets >=1 chunk

    # sub-list start slots in layout order [g][blk][b in g]
    sub_start = np.zeros((NB, NBLK), dtype=np.int64)
    pos = 0
    for gi in range(NG):
        bks = range(gi * GROUP, min((gi + 1) * GROUP, NB))
        for bi in range(NBLK):
            for b in bks:
                sub_start[b, bi] = pos
                pos += Cmat[b, bi] * P
    T = pos // P                            # total chunks per core

    # per-edge slot assignment
    sort_key = ((core * NG + g) * NBLK + blk) * GROUP + bing
    perm = np.argsort(sort_key, kind="stable")
    rid = sort_key[perm]
    starts = np.r_[0, np.flatnonzero(np.diff(rid)) + 1]
    counts = np.diff(np.r_[starts, E])
    rank = np.arange(E, dtype=np.int64) - np.repeat(starts, counts)
    slot = sub_start[bucket[perm], blk[perm]] + rank
    core_p = core[perm]

    src_arr = np.zeros((CORES, T * P), dtype=np.int16)
    dst_arr = np.full((CORES, T * P), -1.0, dtype=np.float32)
    src_arr[core_p, slot] = srcl[perm]
    dst_arr[core_p, slot] = din[perm]

    idx_np = np.empty((CORES, P, T * 8), dtype=np.int16)
    dstv_np = np.empty((CORES, P, T), dtype=np.float32)
    for c in range(CORES):
        w = src_arr[c].reshape(T * 8, 16).T          # [16, 8T]
        idx_np[c] = np.tile(w, (8, 1))
        dstv_np[c] = dst_arr[c].reshape(T, P).T      # [128, T]

    meta = dict(NPC=NPC, NB=NB, NBLK=NBLK, NG=NG, T=T, sub_start=sub_start)
    return Cmat, meta, idx_np, dstv_np


def build_nc(Cmat, meta, cfg):
    N, D, CORES, BLOCK_ROWS, GROUP = (
        cfg["N"], cfg["D"], cfg["CORES"], cfg["BLOCK_ROWS"], cfg["GROUP"])
    NB, NBLK, NG, T = meta["NB"], meta["NBLK"], meta["NG"], meta["T"]
    sub_start = meta["sub_start"]
    f32 = mybir.dt.float32

    # bucket -> (first,last) chunk indices for start/stop flags
    first_chunk = {}
    last_chunk = {}
    for b in range(NB):
        chunks = []
        for bi in range(NBLK):
            s0 = sub_start[b, bi] // P
            chunks.extend(range(s0, s0 + Cmat[b, bi]))
        first_chunk[b] = min(chunks)
        last_chunk[b] = max(chunks)

    nc = bacc.Bacc(None, target_bir_lowering=False)
    x = nc.dram_tensor("x", [N, D], f32, kind="ExternalInput")
    idx_in = nc.dram_tensor("idx", [P, T * 8], mybir.dt.int16, kind="ExternalInput")
    dstv_in = nc.dram_tensor("dstv", [P, T], f32, kind="ExternalInput")
    iota_in = nc.dram_tensor("iota", [P, P], f32, kind="ExternalInput")
    out = nc.dram_tensor("out", [D, NB * P], f32, kind="ExternalOutput")

    with tile.TileContext(nc) as tc:
        with (
            tc.tile_pool(name="persist", bufs=1) as persist,
            tc.tile_pool(name="stag", bufs=3) as stagp,
            tc.tile_pool(name="oh", bufs=2) as ohp,
            tc.tile_pool(name="psum", bufs=8, space="PSUM") as psump,
        ):
            idx_t = persist.tile([P, T * 8], mybir.dt.int16)
            dstv_t = persist.tile([P, T], f32)
            iota_t = persist.tile([P, P], f32)
            outst = persist.tile([D, NB * P], f32)
            nc.sync.dma_start(idx_t[:], idx_in[:])
            nc.sync.dma_start(dstv_t[:], dstv_in[:])
            nc.sync.dma_start(iota_t[:], iota_in[:])

            for gi in range(NG):
                bks = list(range(gi * GROUP, min((gi + 1) * GROUP, NB)))
                # one psum tile (= one bank) per bucket
                ptiles = []
                for h in range(len(bks)):
                    pt_tile = psump.tile([D, P], f32, tag="ps", name=f"ps_{gi}_{h}")
                    ptiles.append(pt_tile)

                for bi in range(NBLK):
                    nck = int(sum(Cmat[b, bi] for b in bks))
                    if nck == 0:
                        continue
                    s0 = int(sub_start[bks[0], bi])      # slot offset
                    t0 = s0 // P                          # chunk offset
                    stag = stagp.tile([P, nck, D], f32, tag="st")
                    GMAX = cfg.get("GMAX", 8)
                    for o in range(0, nck, GMAX):
                        w = min(GMAX, nck - o)
                        nc.gpsimd.dma_gather(
                            stag[:, o:o + w, :],
                            x[bi * BLOCK_ROWS:(bi + 1) * BLOCK_ROWS, :],
                            idx_t[:, (t0 + o) * 8:(t0 + o + w) * 8],
                            w * P,
                            w * P,
                            D,
                        )
                    oh = ohp.tile([P, nck, P], f32, tag="oh")
                    nc.vector.tensor_tensor(
                        out=oh[:],
                        in0=dstv_t[:, t0:t0 + nck].to_broadcast([P, nck, P]),
                        in1=iota_t[:, None, :].to_broadcast([P, nck, P]),
                        op=mybir.AluOpType.is_equal,
                    )
                    tl = 0
                    for b in bks:
                        for _c in range(int(Cmat[b, bi])):
                            tchunk = t0 + tl
                            pt = ptiles[b - bks[0]]
                            nc.tensor.matmul(
                                out=pt[:, :],
                                lhsT=stag[:, tl, :],
                                rhs=oh[:, tl, :],
                                start=(tchunk == first_chunk[b]),
                                stop=(tchunk == last_chunk[b]),
                            )
                            tl += 1

                for h, pt in enumerate(ptiles):
                    c0 = (bks[0] + h) * P
                    nc.scalar.copy(out=outst[:, c0:c0 + P], in_=pt[:, :])

            nc.sync.dma_start(out[:], outst[:])
    nc.finalize()
    return nc


_CACHE = {}


def _get_nc(Cmat, meta, cfg):
    key = (Cmat.tobytes(), meta["T"], cfg["N"], cfg["D"], cfg["CORES"])
    if key not in _CACHE:
        _CACHE[key] = build_nc(Cmat, meta, cfg)
    return _CACHE[key]


def make_in_maps(x, idx_np, dstv_np, cfg):
    CORES, D = cfg["CORES"], cfg["D"]
    xf = np.ascontiguousarray(np.asarray(x, dtype=np.float32))
    iota = np.broadcast_to(np.arange(P, dtype=np.float32), (P, P)).copy()
    return [
        {"x": xf, "idx": idx_np[c], "dstv": dstv_np[c], "iota": iota}
        for c in range(CORES)
    ]


def kernel(x, edge_index):
    cfg = full_cfg()
    Cmat, meta, idx_np, dstv_np = make_layout(edge_index, cfg)
    nc = _get_nc(Cmat, meta, cfg)
    in_maps = make_in_maps(x, idx_np, dstv_np, cfg)
    res = run_bass_kernel_spmd(nc, in_maps, core_ids=list(range(cfg["CORES"])))
    NPC = meta["NPC"]
    shards = [res.results[c]["out"][:, :NPC] for c in range(cfg["CORES"])]
    full = np.concatenate(shards, axis=1).T
    return np.ascontiguousarray(full)
